# revision 34
# baseline (speedup 1.0000x reference)
"""Correlation cost-volume kernel for Trainium2 (8 NeuronCores).

out[b,d,h,w] = sum_c left[b,c,h,w] * right[b,c,h,w-shift[d]]
  left/right: [4, 64, 256, 512] f32, shift: arange(96) -> out [4, 96, 256, 512] f32

Strategy (v8 — K=64, 64-wide windows, diagonal PE tile positions):
  - Shard (b, h-half) across 8 cores: per-core left/right [64, 128, 512], no halo
    (shifts are along W only), no collectives.
  - The cost volume is a 96-wide anti-band of the per-h Gram matrix
    G[i, j] = sum_c L[c, wg+i] * R[c, wg-95+j], computed as TensorEngine
    matmuls [K=64, M=64, N=159] in bf16 over 64-wide w-windows (8 per h row).
    Wider windows cut streamed rhs columns/h from 16x127 to 8x159 (the rhs
    stream bus carries one column per cycle, so columns ~ tensor time).
  - Two h rows are packed in partitions 0-63 / 64-127.  MM(k2, par0) sits at
    PE tile position (0, 0), MM(k2, par1) at (64, 64): disjoint diagonal
    subarrays, so the parity pair's LDWEIGHTS pulls ahead (row groups differ)
    and both write one PSUM bank at disjoint partitions.
  - PSUM bank [128, 318] holds two windows x both parities; copies split
    across Vector (windows 0-3) and Scalar (4-7); raw 159-wide Gram rows go
    straight to output DRAM as one clean contiguous DMA per h-pair
    (2544-byte runs).  Input DMAs issue from the gpsimd ring.
  - The band shear (d = i6 + 95 - j) is undone on the HOST with a zero-copy
    as_strided view — no diagonal DMAs, no DRAM scratch.
  - Host: pack/cast inputs to bf16, de-shear + upcast + transpose the output.
"""
import sys

sys.path.insert(0, "/opt/trn_rl_repo")

import numpy as np
import ml_dtypes

import concourse.bass as bass
import concourse.mybir as mybir
import concourse.tile as tile
from concourse.ap import AP
from concourse.bass_utils import run_bass_kernel_spmd
from concourse.vector_clock import ScopedClock

B, C, H, W, D = 4, 64, 256, 512, 96
HC = H // 2          # 128 h rows per core
S = 64               # w-window per matmul
NW = W // S          # 8 windows per h row
NG = S + D - 1       # 159 gram columns per window
BLK = 64             # h rows per block
NBLK = HC // BLK     # 8 blocks
PAIR_COLS = (D - 1) + W + W  # 95 pad + 512 R + 512 L = 1119
R_OFF = D - 1        # R data starts at col 95 within a pair's R region
L_OFF = (D - 1) + W  # L data starts at col 607
ROW = NW * NG        # out cols per h-pair: (k2, j) = 8*159 = 1272

BF16 = mybir.dt.bfloat16
F32 = mybir.dt.float32


_orig_add_instruction = tile.TileContext._add_instruction


def _patched_add_instruction(self, inst):
    # This walrus build allows at most ONE sync-wait per instruction: peel
    # extra waits onto single-wait NOPs on the same engine, just before it.
    si = inst.sync_info
    if si is not None and len(si.on_wait) > 1:
        waits = list(si.on_wait)
        for w in waits[:-1]:
            nop = mybir.InstNoOp(
                name=self.nc.get_next_instruction_name(),
                text_hint="split_wait",
                bass_nofuse=True,
            )
            nop.engine = inst.engine
            nop.sync_info = mybir.SyncInfo(on_wait=[w], on_update=[])
            _orig_add_instruction(self, nop)
        si.on_wait = waits[-1:]
    _orig_add_instruction(self, inst)


tile.TileContext._add_instruction = _patched_add_instruction


def _patched_drain_and_barrier(self, tick_clock, wait_clock):
    # This walrus build allows only ONE sync-wait on the tail Drain CTRL
    # instruction; split the final-clock waits across single-wait NOPs.
    nc = self.nc
    probe = nc.sync.nop(nofuse=True, hint="drain_waits")
    wait_clock.add_sem_waits(probe.ins, ScopedClock({None: tick_clock.global_clock}))
    waits = list(probe.ins.sync_info.on_wait)
    probe.ins.sync_info.on_wait = waits[:1]
    for w in waits[1:]:
        n = nc.sync.nop(nofuse=True, hint="drain_waits")
        n.ins.sync_info = mybir.SyncInfo(on_wait=[w], on_update=[])
    nc.sync.drain()
    nc.all_engine_barrier()
    assert self.sems is not None
    popped = nc._tile_sem_poison_stack.pop()
    assert popped is self._sem_poison
    nc.clear_and_free_semaphores(list(self.sems.allocated().values()))
    nc.all_engine_barrier()


tile.TileContext._drain_and_barrier = _patched_drain_and_barrier


def build_graph():
    nc = bass.Bass()
    lr_ext = nc.declare_dram_parameter("lrpack", [128, HC // 2, 2 * W], BF16, isOutput=False)
    # raw (sheared) band rows: [h-pair, partition(=64*par+i6), (k2, j)]
    out_ext = nc.declare_dram_parameter("out", [HC // 2, 128, ROW], BF16, isOutput=True)

    with tile.TileContext(nc) as tc:
        IN_BUFS = 2
        with (
            tc.tile_pool(name="inp", bufs=IN_BUFS) as in_pool,
            tc.tile_pool(name="outsb", bufs=8) as out_pool,
            tc.tile_pool(name="psum", bufs=8, space="PSUM") as psum_pool,
        ):
            warm_ps = psum_pool.tile([128, W], F32, tag="warm", bufs=1)
            for blk in range(NBLK):
                # ---- load one block: 8 h-pairs -------------------------------
                blk_tile = in_pool.tile([128, (BLK // 2) * PAIR_COLS], BF16)
                pitch = blk_tile.tensor.shape[1]
                # The 95-col pad is never overwritten (input DMAs touch
                # disjoint cols), so zero each rotating slot exactly once.
                if blk < IN_BUFS:
                    pad_ap = AP(
                        tensor=blk_tile.tensor,
                        offset=blk_tile.offset,
                        ap=[[pitch, 128], [PAIR_COLS, BLK // 2], [1, R_OFF]],
                    )
                    nc.vector.memset(pad_ap, 0.0)
                h2_0 = blk * (BLK // 2)
                # input DMAs on the gpsimd ring (input never queues behind
                # output DMAs, which issue from the sync ring); 2048-B runs.
                # Block 0 splits per-pair so the first matmuls start after one
                # 262KB slice (~0.7us) instead of the whole 4.2MB block
                # (~7us of serial head); later blocks batch one DMA each
                # since their completion latency hides behind compute.
                if blk == 0:
                    for j2 in range(BLK // 2):
                        dst_p = AP(
                            tensor=blk_tile.tensor,
                            offset=blk_tile.offset + j2 * PAIR_COLS + R_OFF,
                            ap=[[pitch, 128], [1, 2 * W]],
                        )
                        nc.gpsimd.dma_start(dst_p, lr_ext[:, h2_0 + j2, :])
                else:
                    dst_rl = AP(
                        tensor=blk_tile.tensor,
                        offset=blk_tile.offset + R_OFF,
                        ap=[[pitch, 128], [PAIR_COLS, BLK // 2], [1, 2 * W]],
                    )
                    nc.gpsimd.dma_start(dst_rl, lr_ext[:, h2_0 : h2_0 + BLK // 2, :])

                # ---- compute: per h-pair, 16 matmuls (8 windows x 2 par) -----
                # PSUM: three tiles per pair (windows 0-2 / 3-5 / 6-7) leave
                # one bank free as a HAM keep-alive target: dependency-free
                # dummy matmuls between pairs fill the PE idle gaps so the
                # clock gate stays at 8/8 (measured 64us of K=4/8 otherwise).
                for j2 in range(BLK // 2):
                    base = j2 * PAIR_COLS
                    out_sb = out_pool.tile([128, ROW], BF16)
                    psA = psum_pool.tile([128, 3 * NG], F32, tag="ps", bufs=7)
                    psB = psum_pool.tile([128, 3 * NG], F32, tag="ps", bufs=7)
                    psC = psum_pool.tile([128, 3 * NG], F32, tag="ps", bufs=7)
                    tiles = (psA, psB, psC)
                    for k2 in range(NW):
                        ps = tiles[k2 // 3]
                        col = (k2 % 3) * NG
                        wg = S * k2
                        # par0 at PE position (0,0) -> partitions 0-63,
                        # par1 at (64,64) -> partitions 64-127: disjoint
                        # diagonal subarrays, LDWEIGHTS pulls ahead
                        for par in range(2):
                            p0 = 64 * par
                            lhsT = blk_tile[p0 : p0 + 64, base + L_OFF + wg : base + L_OFF + wg + S]
                            rhs = blk_tile[p0 : p0 + 64, base + wg : base + wg + NG]
                            nc.tensor.matmul(
                                ps[p0 : p0 + 64, col : col + NG],
                                lhsT=lhsT,
                                rhs=rhs,
                                start=True,
                                stop=True,
                                tile_position=(p0, p0),
                            )
                        # copy each tile as soon as its windows are done.
                        # ALL of a pair's copies go on ONE engine, alternating
                        # per pair (coarse per-engine-clock WAR waits -> the
                        # next pair only waits on the same-parity pair).
                        if k2 in (2, 5, 7):
                            ti = k2 // 3
                            w0 = 3 * ti
                            nwin = k2 - w0 + 1
                            dst = out_sb[:, w0 * NG : (w0 + nwin) * NG]
                            if (blk * (BLK // 2) + j2) % 2 == 0:
                                nc.vector.tensor_copy(dst, tiles[ti][:, 0 : nwin * NG])
                            else:
                                nc.scalar.copy(dst, tiles[ti][:, 0 : nwin * NG])
                    # keep-alive: two dependency-free dummy matmuls into the
                    # spare bank bridge the inter-pair PE gap (never read)
                    for par in range(2):
                        p0 = 64 * par
                        nc.tensor.matmul(
                            warm_ps[p0 : p0 + 64, 0:W],
                            lhsT=blk_tile[p0 : p0 + 64, base : base + S],
                            rhs=blk_tile[p0 : p0 + 64, base : base + W],
                            start=True,
                            stop=True,
                            tile_position=(p0, p0),
                        )
                    # one clean DMA per pair: contiguous 2544-byte runs
                    # (dma_start ring-issue costs ~0.6us each — keep one)
                    nc.sync.dma_start(out_ext[blk * (BLK // 2) + j2], out_sb[:])
    return nc


_CACHED = {}


def _get_graph():
    if "nc" not in _CACHED:
        _CACHED["nc"] = build_graph()
    return _CACHED["nc"]


def _pack_core(left_b, right_b, h0):
    """left_b/right_b: [C, H, W] f32 for one batch -> lrpack [128, 64, 1024] bf16.

    Layout: R row then L row contiguously (SBUF gets [pad|R|L] in one DMA);
    h-parity on partition halves (even h -> partitions 0-63, odd -> 64-127).
    """
    ls = left_b[:, h0 : h0 + HC, :]
    rs = right_b[:, h0 : h0 + HC, :]
    pack = np.empty((128, HC // 2, 2 * W), dtype=np.float32)
    pack[0:64, :, 0:W] = rs[:, 0::2, :]
    pack[64:128, :, 0:W] = rs[:, 1::2, :]
    pack[0:64, :, W : 2 * W] = ls[:, 0::2, :]
    pack[64:128, :, W : 2 * W] = ls[:, 1::2, :]
    return pack.astype(ml_dtypes.bfloat16)


def _unshear_core(oc):
    """oc: [64, 128, 1272] bf16 raw band rows -> [D, HC, W] f32.

    raw[h2, p=64*par+i6, k2*159 + j] = G at w = 64*k2 + i6, h = 2*h2 + par,
    d = i6 + 95 - j.  De-shear with a strided view: j = i6 + 95 - d.
    """
    r5 = oc.reshape(64, 2, S, NW, NG)  # [h2, par, i6, k2, j]
    s = r5.strides
    v = np.lib.stride_tricks.as_strided(
        r5[:, :, :, :, 95:],
        shape=(64, 2, S, NW, D),
        strides=(s[0], s[1], s[2] + s[4], s[3], -s[4]),
    )
    # v dims: [h2, par, i6, k2, d] -> [d, (h2, par), (k2, i6)]
    return v.transpose(4, 0, 1, 3, 2).reshape(D, HC, W).astype(np.float32)


def _run(inputs, trace=False):
    left = np.asarray(inputs["left"], dtype=np.float32)
    right = np.asarray(inputs["right"], dtype=np.float32)
    shift = np.asarray(inputs["shift"])

    nc = _get_graph()
    in_maps = []
    for core in range(8):
        b, half = core // 2, core % 2
        in_maps.append({"lrpack": _pack_core(left[b], right[b], half * HC)})

    res = run_bass_kernel_spmd(nc, in_maps, core_ids=list(range(8)), trace=trace)

    out = np.empty((B, D, H, W), dtype=np.float32)
    for core in range(8):
        b, half = core // 2, core % 2
        oc = np.asarray(res.results[core]["out"])  # [64, 128, 1272] bf16
        out[b, :, half * HC : (half + 1) * HC, :] = _unshear_core(oc)

    # band covers integer shifts 0..95; remap if shift isn't exactly arange
    s = np.asarray(shift, dtype=np.float64)
    if not np.allclose(s, np.arange(D)):
        si = np.rint(s).astype(np.int64)
        if np.allclose(s, si) and si.min() >= 0 and si.max() < D:
            out = out[:, si, :, :]
        else:
            raise NotImplementedError(f"unsupported shift vector: {s}")
    return out, res


def kernel(**inputs) -> np.ndarray:
    out, _ = _run(inputs, trace=False)
    return out


# revision 36
# speedup vs baseline: 1.0103x; 1.0103x over previous
"""Correlation cost-volume kernel for Trainium2 (8 NeuronCores).

out[b,d,h,w] = sum_c left[b,c,h,w] * right[b,c,h,w-shift[d]]
  left/right: [4, 64, 256, 512] f32, shift: arange(96) -> out [4, 96, 256, 512] f32

Strategy (v8 — K=64, 64-wide windows, diagonal PE tile positions):
  - Shard (b, h-half) across 8 cores: per-core left/right [64, 128, 512], no halo
    (shifts are along W only), no collectives.
  - The cost volume is a 96-wide anti-band of the per-h Gram matrix
    G[i, j] = sum_c L[c, wg+i] * R[c, wg-95+j], computed as TensorEngine
    matmuls [K=64, M=64, N=159] in bf16 over 64-wide w-windows (8 per h row).
    Wider windows cut streamed rhs columns/h from 16x127 to 8x159 (the rhs
    stream bus carries one column per cycle, so columns ~ tensor time).
  - Two h rows are packed in partitions 0-63 / 64-127.  MM(k2, par0) sits at
    PE tile position (0, 0), MM(k2, par1) at (64, 64): disjoint diagonal
    subarrays, so the parity pair's LDWEIGHTS pulls ahead (row groups differ)
    and both write one PSUM bank at disjoint partitions.
  - PSUM bank [128, 318] holds two windows x both parities; copies split
    across Vector (windows 0-3) and Scalar (4-7); raw 159-wide Gram rows go
    straight to output DRAM as one clean contiguous DMA per h-pair
    (2544-byte runs).  Input DMAs issue from the gpsimd ring.
  - The band shear (d = i6 + 95 - j) is undone on the HOST with a zero-copy
    as_strided view — no diagonal DMAs, no DRAM scratch.
  - Host: pack/cast inputs to bf16, de-shear + upcast + transpose the output.
"""
import sys

sys.path.insert(0, "/opt/trn_rl_repo")

import numpy as np
import ml_dtypes

import concourse.bass as bass
import concourse.mybir as mybir
import concourse.tile as tile
from concourse.ap import AP
from concourse.bass_utils import run_bass_kernel_spmd
from concourse.vector_clock import ScopedClock

B, C, H, W, D = 4, 64, 256, 512, 96
HC = H // 2          # 128 h rows per core
S = 64               # w-window per matmul
NW = W // S          # 8 windows per h row
NG = S + D - 1       # 159 gram columns per window
BLK = 32             # h rows per block
NBLK = HC // BLK     # 8 blocks
PAIR_COLS = (D - 1) + W + W  # 95 pad + 512 R + 512 L = 1119
R_OFF = D - 1        # R data starts at col 95 within a pair's R region
L_OFF = (D - 1) + W  # L data starts at col 607
ROW = NW * NG        # out cols per h-pair: (k2, j) = 8*159 = 1272

BF16 = mybir.dt.bfloat16
F32 = mybir.dt.float32


_orig_add_instruction = tile.TileContext._add_instruction


def _patched_add_instruction(self, inst):
    # This walrus build allows at most ONE sync-wait per instruction: peel
    # extra waits onto single-wait NOPs on the same engine, just before it.
    si = inst.sync_info
    if si is not None and len(si.on_wait) > 1:
        waits = list(si.on_wait)
        for w in waits[:-1]:
            nop = mybir.InstNoOp(
                name=self.nc.get_next_instruction_name(),
                text_hint="split_wait",
                bass_nofuse=True,
            )
            nop.engine = inst.engine
            nop.sync_info = mybir.SyncInfo(on_wait=[w], on_update=[])
            _orig_add_instruction(self, nop)
        si.on_wait = waits[-1:]
    _orig_add_instruction(self, inst)


tile.TileContext._add_instruction = _patched_add_instruction


def _patched_drain_and_barrier(self, tick_clock, wait_clock):
    # This walrus build allows only ONE sync-wait on the tail Drain CTRL
    # instruction; split the final-clock waits across single-wait NOPs.
    nc = self.nc
    probe = nc.sync.nop(nofuse=True, hint="drain_waits")
    wait_clock.add_sem_waits(probe.ins, ScopedClock({None: tick_clock.global_clock}))
    waits = list(probe.ins.sync_info.on_wait)
    probe.ins.sync_info.on_wait = waits[:1]
    for w in waits[1:]:
        n = nc.sync.nop(nofuse=True, hint="drain_waits")
        n.ins.sync_info = mybir.SyncInfo(on_wait=[w], on_update=[])
    nc.sync.drain()
    nc.all_engine_barrier()
    assert self.sems is not None
    popped = nc._tile_sem_poison_stack.pop()
    assert popped is self._sem_poison
    nc.clear_and_free_semaphores(list(self.sems.allocated().values()))
    nc.all_engine_barrier()


tile.TileContext._drain_and_barrier = _patched_drain_and_barrier


def build_graph():
    nc = bass.Bass()
    lr_ext = nc.declare_dram_parameter("lrpack", [128, HC // 2, 2 * W], BF16, isOutput=False)
    # raw (sheared) band rows: [h-pair, partition(=64*par+i6), (k2, j)]
    out_ext = nc.declare_dram_parameter("out", [HC // 2, 128, ROW], BF16, isOutput=True)

    with tile.TileContext(nc) as tc:
        IN_BUFS = 3
        with (
            tc.tile_pool(name="inp", bufs=IN_BUFS) as in_pool,
            tc.tile_pool(name="outsb", bufs=10) as out_pool,
            tc.tile_pool(name="psum", bufs=8, space="PSUM") as psum_pool,
        ):
            warm_ps = psum_pool.tile([128, W], F32, tag="warm", bufs=1)
            for blk in range(NBLK):
                # ---- load one block: 8 h-pairs -------------------------------
                blk_tile = in_pool.tile([128, (BLK // 2) * PAIR_COLS], BF16)
                pitch = blk_tile.tensor.shape[1]
                # The 95-col pad is never overwritten (input DMAs touch
                # disjoint cols), so zero each rotating slot exactly once.
                if blk < IN_BUFS:
                    pad_ap = AP(
                        tensor=blk_tile.tensor,
                        offset=blk_tile.offset,
                        ap=[[pitch, 128], [PAIR_COLS, BLK // 2], [1, R_OFF]],
                    )
                    nc.vector.memset(pad_ap, 0.0)
                h2_0 = blk * (BLK // 2)
                # input DMAs on the gpsimd ring (input never queues behind
                # output DMAs, which issue from the sync ring); 2048-B runs.
                # Block 0 splits per-pair so the first matmuls start after one
                # 262KB slice (~0.7us) instead of the whole 4.2MB block
                # (~7us of serial head); later blocks batch one DMA each
                # since their completion latency hides behind compute.
                if blk == 0:
                    for j2 in range(BLK // 2):
                        dst_p = AP(
                            tensor=blk_tile.tensor,
                            offset=blk_tile.offset + j2 * PAIR_COLS + R_OFF,
                            ap=[[pitch, 128], [1, 2 * W]],
                        )
                        nc.gpsimd.dma_start(dst_p, lr_ext[:, h2_0 + j2, :])
                else:
                    dst_rl = AP(
                        tensor=blk_tile.tensor,
                        offset=blk_tile.offset + R_OFF,
                        ap=[[pitch, 128], [PAIR_COLS, BLK // 2], [1, 2 * W]],
                    )
                    nc.gpsimd.dma_start(dst_rl, lr_ext[:, h2_0 : h2_0 + BLK // 2, :])

                # ---- compute: per h-pair, 16 matmuls (8 windows x 2 par) -----
                # PSUM: three tiles per pair (windows 0-2 / 3-5 / 6-7) leave
                # one bank free as a HAM keep-alive target: dependency-free
                # dummy matmuls between pairs fill the PE idle gaps so the
                # clock gate stays at 8/8 (measured 64us of K=4/8 otherwise).
                for j2 in range(BLK // 2):
                    base = j2 * PAIR_COLS
                    out_sb = out_pool.tile([128, ROW], BF16)
                    psA = psum_pool.tile([128, 3 * NG], F32, tag="ps", bufs=7)
                    psB = psum_pool.tile([128, 3 * NG], F32, tag="ps", bufs=7)
                    psC = psum_pool.tile([128, 3 * NG], F32, tag="ps", bufs=7)
                    tiles = (psA, psB, psC)
                    for k2 in range(NW):
                        ps = tiles[k2 // 3]
                        col = (k2 % 3) * NG
                        wg = S * k2
                        # par0 at PE position (0,0) -> partitions 0-63,
                        # par1 at (64,64) -> partitions 64-127: disjoint
                        # diagonal subarrays, LDWEIGHTS pulls ahead
                        for par in range(2):
                            p0 = 64 * par
                            lhsT = blk_tile[p0 : p0 + 64, base + L_OFF + wg : base + L_OFF + wg + S]
                            rhs = blk_tile[p0 : p0 + 64, base + wg : base + wg + NG]
                            nc.tensor.matmul(
                                ps[p0 : p0 + 64, col : col + NG],
                                lhsT=lhsT,
                                rhs=rhs,
                                start=True,
                                stop=True,
                                tile_position=(p0, p0),
                            )
                        # copy each tile as soon as its windows are done.
                        # ALL of a pair's copies go on ONE engine, alternating
                        # per pair (coarse per-engine-clock WAR waits -> the
                        # next pair only waits on the same-parity pair).
                        if k2 in (2, 5, 7):
                            ti = k2 // 3
                            w0 = 3 * ti
                            nwin = k2 - w0 + 1
                            dst = out_sb[:, w0 * NG : (w0 + nwin) * NG]
                            if (blk * (BLK // 2) + j2) % 2 == 0:
                                nc.vector.tensor_copy(dst, tiles[ti][:, 0 : nwin * NG])
                            else:
                                nc.scalar.copy(dst, tiles[ti][:, 0 : nwin * NG])
                    # keep-alive: two dependency-free dummy matmuls into the
                    # spare bank bridge the inter-pair PE gap (never read)
                    for par in range(2):
                        p0 = 64 * par
                        nc.tensor.matmul(
                            warm_ps[p0 : p0 + 64, 0:W],
                            lhsT=blk_tile[p0 : p0 + 64, base : base + S],
                            rhs=blk_tile[p0 : p0 + 64, base : base + W],
                            start=True,
                            stop=True,
                            tile_position=(p0, p0),
                        )
                    # one clean DMA per pair: contiguous 2544-byte runs
                    # (dma_start ring-issue costs ~0.6us each — keep one)
                    nc.sync.dma_start(out_ext[blk * (BLK // 2) + j2], out_sb[:])
    return nc


_CACHED = {}


def _get_graph():
    if "nc" not in _CACHED:
        _CACHED["nc"] = build_graph()
    return _CACHED["nc"]


def _pack_core(left_b, right_b, h0):
    """left_b/right_b: [C, H, W] f32 for one batch -> lrpack [128, 64, 1024] bf16.

    Layout: R row then L row contiguously (SBUF gets [pad|R|L] in one DMA);
    h-parity on partition halves (even h -> partitions 0-63, odd -> 64-127).
    """
    ls = left_b[:, h0 : h0 + HC, :]
    rs = right_b[:, h0 : h0 + HC, :]
    pack = np.empty((128, HC // 2, 2 * W), dtype=np.float32)
    pack[0:64, :, 0:W] = rs[:, 0::2, :]
    pack[64:128, :, 0:W] = rs[:, 1::2, :]
    pack[0:64, :, W : 2 * W] = ls[:, 0::2, :]
    pack[64:128, :, W : 2 * W] = ls[:, 1::2, :]
    return pack.astype(ml_dtypes.bfloat16)


def _unshear_core(oc):
    """oc: [64, 128, 1272] bf16 raw band rows -> [D, HC, W] f32.

    raw[h2, p=64*par+i6, k2*159 + j] = G at w = 64*k2 + i6, h = 2*h2 + par,
    d = i6 + 95 - j.  De-shear with a strided view: j = i6 + 95 - d.
    """
    r5 = oc.reshape(64, 2, S, NW, NG)  # [h2, par, i6, k2, j]
    s = r5.strides
    v = np.lib.stride_tricks.as_strided(
        r5[:, :, :, :, 95:],
        shape=(64, 2, S, NW, D),
        strides=(s[0], s[1], s[2] + s[4], s[3], -s[4]),
    )
    # v dims: [h2, par, i6, k2, d] -> [d, (h2, par), (k2, i6)]
    return v.transpose(4, 0, 1, 3, 2).reshape(D, HC, W).astype(np.float32)


def _run(inputs, trace=False):
    left = np.asarray(inputs["left"], dtype=np.float32)
    right = np.asarray(inputs["right"], dtype=np.float32)
    shift = np.asarray(inputs["shift"])

    nc = _get_graph()
    in_maps = []
    for core in range(8):
        b, half = core // 2, core % 2
        in_maps.append({"lrpack": _pack_core(left[b], right[b], half * HC)})

    res = run_bass_kernel_spmd(nc, in_maps, core_ids=list(range(8)), trace=trace)

    out = np.empty((B, D, H, W), dtype=np.float32)
    for core in range(8):
        b, half = core // 2, core % 2
        oc = np.asarray(res.results[core]["out"])  # [64, 128, 1272] bf16
        out[b, :, half * HC : (half + 1) * HC, :] = _unshear_core(oc)

    # band covers integer shifts 0..95; remap if shift isn't exactly arange
    s = np.asarray(shift, dtype=np.float64)
    if not np.allclose(s, np.arange(D)):
        si = np.rint(s).astype(np.int64)
        if np.allclose(s, si) and si.min() >= 0 and si.max() < D:
            out = out[:, si, :, :]
        else:
            raise NotImplementedError(f"unsupported shift vector: {s}")
    return out, res


def kernel(**inputs) -> np.ndarray:
    out, _ = _run(inputs, trace=False)
    return out
